# revision 7
# baseline (speedup 1.0000x reference)
"""Bass/Trainium2 kernel for the bidirectional-LSTM discriminator.

Sharding: 8 cores = 4 batch-slices x 2 directions (data-parallel on batch;
the reverse direction runs the same program on time-flipped input).
Each core: MLP (feature-major GEMMs) -> x3^T resident in SBUF ->
LSTM recurrence with gates accumulated in PSUM banks (i2h GEMM and h2h
matmuls accumulate into the same bank; biases enter via a K=8 indicator
matmul). All transcendentals are sigmoids (tanh folded as 2*sigmoid(2x)-1
with the scale-by-2 folded into weights host-side; q is kept halved on
device with wh pre-doubled to compensate).
"""

import sys

sys.path.insert(0, "/opt/trn_rl_repo")

import numpy as np  # noqa: E402

import concourse.bass as bass  # noqa: E402
import concourse.bacc as bacc  # noqa: E402
import concourse.mybir as mybir  # noqa: E402
import concourse.tile as tile  # noqa: E402
from concourse.bass_utils import run_bass_kernel_spmd  # noqa: E402

F16 = mybir.dt.float16
F32 = mybir.dt.float32
AF = mybir.ActivationFunctionType
ALU = mybir.AluOpType

B, T, HD = 256, 512, 256
NREAL, NCAT, NCLS, ESZ = 8, 4, 10, 8
FEAT = NREAL + NCAT * NCLS  # 48
G4 = 4  # 4H = 1024
B2 = B // 4  # 64 batch per core
NTOK = B2 * T  # 32768 tokens per core
BLK = 512  # MLP token block
NBLK = NTOK // BLK
GRP = 4  # recurrence steps per i2h group (8 PSUM banks = 2 groups in flight)
ALPHA = 0.1  # leaky-relu slope


def _build_program():
    nc = bacc.Bacc("TRN2", target_bir_lowering=False, debug=False)

    x0t = nc.dram_tensor("x0t", [FEAT, NTOK], F16, kind="ExternalInput").ap()
    w01 = nc.dram_tensor("w01", [FEAT, HD], F16, kind="ExternalInput").ap()
    w2d = nc.dram_tensor("w2d", [HD, HD], F16, kind="ExternalInput").ap()
    wid = nc.dram_tensor("wid", [HD, 4 * HD], F16, kind="ExternalInput").ap()
    whd = nc.dram_tensor("whd", [HD, 4 * HD], F16, kind="ExternalInput").ap()
    brow = nc.dram_tensor("brow", [8, 128], F16, kind="ExternalInput").ap()
    ind = nc.dram_tensor("ind", [8, 512], F16, kind="ExternalInput").ap()
    bact = nc.dram_tensor("bact", [128, 4], F32, kind="ExternalInput").ap()
    qout = nc.dram_tensor("qout", [128, 128], F32, kind="ExternalOutput").ap()

    H4 = 4 * HD  # 1024

    with tile.TileContext(nc) as tc:
        with (
            tc.tile_pool(name="const", bufs=1) as const,
            tc.tile_pool(name="x3pool", bufs=1) as x3pool,
        ):
            # Dummy activation first: pulls the (single) act-table load to
            # kernel start where the instruction has at most one wait.
            dum = const.tile([1, 2], F32)
            nc.vector.memset(dum[:], 0.0)
            nc.scalar.activation(dum[:], dum[:], AF.Sigmoid)

            w01_s = const.tile([FEAT, HD], F16)
            nc.sync.dma_start(w01_s[:], w01)
            w2_s = const.tile([128, 2 * HD], F16)
            wi_s = const.tile([128, 2 * H4], F16)
            wh_s = const.tile([128, 2 * H4], F16)
            for k in range(2):
                nc.sync.dma_start(
                    w2_s[:, k * HD : (k + 1) * HD], w2d[k * 128 : (k + 1) * 128, :]
                )
                nc.sync.dma_start(
                    wi_s[:, k * H4 : (k + 1) * H4], wid[k * 128 : (k + 1) * 128, :]
                )
                nc.sync.dma_start(
                    wh_s[:, k * H4 : (k + 1) * H4], whd[k * 128 : (k + 1) * 128, :]
                )
            brow_s = const.tile([8, 128], F16)
            nc.sync.dma_start(brow_s[:], brow)
            ind_s = const.tile([8, 512], F16)
            nc.sync.dma_start(ind_s[:], ind)
            bact_s = const.tile([128, 4], F32)
            nc.sync.dma_start(bact_s[:], bact)

            # x3^T resident: chunk c (hidden c*128..) at cols [c*NTOK, (c+1)*NTOK)
            x3t = x3pool.tile([128, 2 * NTOK], F16)

            # ---------------- MLP: x0 -> x2 -> x3 (feature-major) ----------
            with (
                tc.tile_pool(name="x0p", bufs=3) as x0p,
                tc.tile_pool(name="x2p", bufs=4) as x2p,
                tc.tile_pool(name="ps1", bufs=2, space="PSUM") as ps1,
                tc.tile_pool(name="ps2", bufs=2, space="PSUM") as ps2,
            ):
                for blk in range(NBLK):
                    x0b = x0p.tile([FEAT, BLK], F16)
                    nc.sync.dma_start(x0b[:], x0t[:, blk * BLK : (blk + 1) * BLK])
                    x2b = []
                    for c in range(2):
                        p1 = ps1.tile([128, BLK], F32)
                        nc.tensor.matmul(
                            p1[:],
                            w01_s[:, c * 128 : (c + 1) * 128],
                            x0b[:],
                            start=True,
                            stop=True,
                        )
                        x2c = x2p.tile([128, BLK], F16)
                        nc.scalar.activation(
                            x2c[:],
                            p1[:],
                            AF.Prelu,
                            bias=bact_s[:, c : c + 1],
                            scale=1.0,
                            alpha=ALPHA,
                        )
                        x2b.append(x2c)
                    for c in range(2):
                        p2 = ps2.tile([128, BLK], F32)
                        for k in range(2):
                            nc.tensor.matmul(
                                p2[:],
                                w2_s[:, k * HD + c * 128 : k * HD + (c + 1) * 128],
                                x2b[k][:],
                                start=(k == 0),
                                stop=(k == 1),
                            )
                        nc.scalar.activation(
                            x3t[:, c * NTOK + blk * BLK : c * NTOK + (blk + 1) * BLK],
                            p2[:],
                            AF.Prelu,
                            bias=bact_s[:, 2 + c : 3 + c],
                            scale=1.0,
                            alpha=ALPHA,
                        )

            # Collapse the vector clock so recurrence instructions don't
            # accumulate waits on every DMA queue used above.
            tc.strict_bb_all_engine_barrier()

            # ---------------- LSTM recurrence ------------------------------
            # bank(t) [128, 512]: gate chunk m at cols m*64.. ; chunk order
            # [F0 F1 I0 I1 O0 O1 A0 A1]. sigma output slices: F=[0:128],
            # I=[128:256], O=[256:384], A=[384:512].
            with (
                tc.tile_pool(name="gbank", bufs=8, space="PSUM") as gb,
                tc.tile_pool(name="sigp", bufs=3) as sigp,
                tc.tile_pool(name="vp", bufs=2) as vp,
                tc.tile_pool(name="v2p", bufs=2) as v2p,
                tc.tile_pool(name="sp", bufs=2) as sp,
                tc.tile_pool(name="s2p", bufs=2) as s2p,
                tc.tile_pool(name="qp", bufs=2) as qp,
                tc.tile_pool(name="outp", bufs=1) as outp,
            ):
                s_prev = sp.tile([128, 128], F32)
                nc.vector.memset(s_prev[:], 0.0)
                qh_prev = qp.tile([128, 128], F16)
                nc.vector.memset(qh_prev[:], 0.0)

                banks = {}
                for g in range(T // GRP):
                    # bias preload + i2h GEMM for this group's banks
                    for j in range(GRP):
                        t = g * GRP + j
                        bk = gb.tile([128, 512], F32)
                        banks[t] = bk
                        nc.tensor.matmul(
                            bk[:], brow_s[:], ind_s[:], start=True, stop=False
                        )
                    for k in range(2):
                        for m in range(8):
                            lhsT = wi_s[:, k * H4 + m * 128 : k * H4 + (m + 1) * 128]
                            for j in range(GRP):
                                t = g * GRP + j
                                nc.tensor.matmul(
                                    banks[t][:, m * 64 : (m + 1) * 64],
                                    lhsT,
                                    x3t[:, k * NTOK + t * 64 : k * NTOK + t * 64 + 64],
                                    start=False,
                                    stop=False,
                                )
                    # serial steps
                    for j in range(GRP):
                        t = g * GRP + j
                        bk = banks.pop(t)
                        for k in range(2):
                            for m in range(8):
                                nc.tensor.matmul(
                                    bk[:, m * 64 : (m + 1) * 64],
                                    wh_s[:, k * H4 + m * 128 : k * H4 + (m + 1) * 128],
                                    qh_prev[:, k * 64 : (k + 1) * 64],
                                    start=False,
                                    stop=(k == 1 and m == 7),
                                )
                        sig = sigp.tile([128, 512], F32)
                        nc.scalar.activation(sig[:], bk[:], AF.Sigmoid)
                        v0 = vp.tile([128, 128], F32)
                        nc.vector.tensor_mul(v0[:], sig[:, 0:128], s_prev[:])
                        v1 = v2p.tile([128, 128], F32)
                        nc.vector.scalar_tensor_tensor(
                            v1[:],
                            sig[:, 384:512],
                            0.5,
                            sig[:, 128:256],
                            op0=ALU.subtract,
                            op1=ALU.mult,
                        )
                        s_new = sp.tile([128, 128], F32)
                        nc.vector.scalar_tensor_tensor(
                            s_new[:], v1[:], 2.0, v0[:], op0=ALU.mult, op1=ALU.add
                        )
                        s2 = s2p.tile([128, 128], F32)
                        nc.scalar.activation(s2[:], s_new[:], AF.Sigmoid, scale=2.0)
                        qh_new = qp.tile([128, 128], F16)
                        nc.vector.scalar_tensor_tensor(
                            qh_new[:],
                            s2[:],
                            0.5,
                            sig[:, 256:384],
                            op0=ALU.subtract,
                            op1=ALU.mult,
                        )
                        s_prev = s_new
                        qh_prev = qh_new
                        if t == T - 1:
                            qf = outp.tile([128, 128], F32)
                            nc.vector.scalar_tensor_tensor(
                                qf[:],
                                s2[:],
                                0.5,
                                sig[:, 256:384],
                                op0=ALU.subtract,
                                op1=ALU.mult,
                            )
                            nc.sync.dma_start(qout, qf[:])
    nc.compile()
    return nc


def _host_prep(x0, emb_w, w1, b1, w2, b2, wi_f, bi_f, wh_f, bh_f, wi_r, bi_r, wh_r, bh_r):
    """Fold weights host-side; build the 8 per-core input maps."""
    f32 = np.float32
    x0 = np.asarray(x0, f32)
    emb_w = np.asarray(emb_w, f32)
    w1, b1 = np.asarray(w1, f32), np.asarray(b1, f32)
    w2, b2 = np.asarray(w2, f32), np.asarray(b2, f32)

    # embedding fold: x1 = x0 @ W0, W0 = blockdiag(I8, emb blocks)
    W0 = np.zeros((FEAT, NREAL + NCAT * ESZ), f32)
    W0[:NREAL, :NREAL] = np.eye(NREAL)
    for c in range(NCAT):
        W0[
            NREAL + c * NCLS : NREAL + (c + 1) * NCLS,
            NREAL + c * ESZ : NREAL + (c + 1) * ESZ,
        ] = emb_w[c]
    W01 = W0 @ w1  # [48, 256]

    # gate-chunk order [F I O A] (orig order F, I, A, O)
    perm = np.r_[0:512, 768:1024, 512:768]

    def prep_dir(wi, bi, wh, bh):
        wi = np.asarray(wi, f32)[:, perm].copy()
        wh = np.asarray(wh, f32)[:, perm].copy()
        bp = (np.asarray(bi, f32) + np.asarray(bh, f32))[perm].copy()
        # tanh(a) = 2*sigmoid(2a)-1: scale A-block (cols 768:1024) by 2
        wi[:, 768:] *= 2.0
        wh[:, 768:] *= 2.0
        bp[768:] *= 2.0
        # device keeps qh = q/2 -> double wh to compensate
        wh *= 2.0
        return wi, wh, bp

    dirs = [prep_dir(wi_f, bi_f, wh_f, bh_f), prep_dir(wi_r, bi_r, wh_r, bh_r)]

    indm = np.zeros((8, 512), np.float16)
    for m in range(8):
        indm[m, m * 64 : (m + 1) * 64] = 1.0
    bactm = np.stack([b1[:128], b1[128:], b2[:128], b2[128:]], axis=1).astype(f32)

    in_maps = []
    for core in range(8):
        d = core // 4
        bsl = slice((core % 4) * B2, (core % 4 + 1) * B2)
        x0c = x0[bsl]  # [64, 512, 48]
        if d == 1:
            x0c = x0c[:, ::-1, :]
        # feature-major, col = t*64 + b
        x0tc = np.ascontiguousarray(x0c.transpose(2, 1, 0)).reshape(FEAT, NTOK)
        wip, whp, bp = dirs[d]
        in_maps.append(
            dict(
                x0t=x0tc.astype(np.float16),
                w01=W01.astype(np.float16),
                w2d=w2.astype(np.float16),
                wid=wip.astype(np.float16),
                whd=whp.astype(np.float16),
                brow=bp.reshape(8, 128).astype(np.float16),
                ind=indm,
                bact=bactm,
            )
        )
    return in_maps


_NC_CACHE = {}


def kernel(
    x0,
    emb_w,
    w1,
    b1,
    w2,
    b2,
    wi_f,
    bi_f,
    wh_f,
    bh_f,
    wi_r,
    bi_r,
    wh_r,
    bh_r,
    w3,
    b3,
):
    in_maps = _host_prep(
        x0, emb_w, w1, b1, w2, b2, wi_f, bi_f, wh_f, bh_f, wi_r, bi_r, wh_r, bh_r
    )
    if "nc" not in _NC_CACHE:
        _NC_CACHE["nc"] = _build_program()
    res = run_bass_kernel_spmd(_NC_CACHE["nc"], in_maps, list(range(8))).results

    q = np.zeros((2, B, HD), np.float32)  # [dir, batch, hid]
    for core in range(8):
        d, bi_ = core // 4, core % 4
        qo = np.asarray(res[core]["qout"], np.float32) * 2.0  # [128, 128]
        for k in range(2):
            q[d, bi_ * B2 : (bi_ + 1) * B2, k * 128 : (k + 1) * 128] = qo[
                :, k * B2 : (k + 1) * B2
            ].T
    x4 = np.concatenate([q[0], q[1]], axis=1)  # [B, 512]
    return (x4 @ np.asarray(w3, np.float32) + np.asarray(b3, np.float32)).astype(
        np.float32
    )


def golden(
    x0,
    emb_w,
    w1,
    b1,
    w2,
    b2,
    wi_f,
    bi_f,
    wh_f,
    bh_f,
    wi_r,
    bi_r,
    wh_r,
    bh_r,
    w3,
    b3,
    quant=False,
):
    """Numpy model of EXACTLY the device math (for host-side validation)."""
    f32 = np.float32

    def q16(a):
        return a.astype(np.float16).astype(f32) if quant else a.astype(f32)

    in_maps = _host_prep(
        x0, emb_w, w1, b1, w2, b2, wi_f, bi_f, wh_f, bh_f, wi_r, bi_r, wh_r, bh_r
    )
    sig = lambda v: 1.0 / (1.0 + np.exp(-v))
    lrelu = lambda v: np.where(v >= 0, v, ALPHA * v)
    q = np.zeros((2, B, HD), f32)
    for core in range(8):
        m = in_maps[core]
        d, bi_ = core // 4, core % 4
        x0tc = q16(m["x0t"].astype(f32))  # [48, NTOK]
        W01 = q16(m["w01"].astype(f32))
        w2c = q16(m["w2d"].astype(f32))
        wip = q16(m["wid"].astype(f32))
        whp = q16(m["whd"].astype(f32))
        bp = m["brow"].astype(f32).reshape(1024)
        b1c = np.concatenate([m["bact"][:, 0], m["bact"][:, 1]])
        b2c = np.concatenate([m["bact"][:, 2], m["bact"][:, 3]])
        x2 = q16(lrelu(W01.T @ x0tc + b1c[:, None]))  # [256, NTOK]
        x3 = q16(lrelu(w2c.T @ x2 + b2c[:, None]))  # [256, NTOK]
        gx = wip.T @ x3 + bp[:, None]  # [1024, NTOK]
        s = np.zeros((HD, B2), f32)
        qh = np.zeros((HD, B2), f32)
        for t in range(T):
            gates = sig(gx[:, t * B2 : (t + 1) * B2] + whp.T @ qh)
            f, i, o, a = gates[:256], gates[256:512], gates[512:768], gates[768:]
            s = f * s + 2.0 * ((a - 0.5) * i)
            s2 = sig(2.0 * s)
            qh = q16((s2 - 0.5) * o)  # q/2
        qfull = 2.0 * qh  # [256, 64]
        q[d, bi_ * B2 : (bi_ + 1) * B2] = qfull.T
    x4 = np.concatenate([q[0], q[1]], axis=1)
    return (x4 @ np.asarray(w3, f32) + np.asarray(b3, f32)).astype(f32)
